# revision 18
# baseline (speedup 1.0000x reference)
"""BiLSTM-CRF forward-algorithm (log-partition) Trainium2 kernel.

Math (exp-domain scaled forward algorithm):
    alpha_{t+1}[b,c] = logsumexp_p(alpha_t[b,p] + trans[p,c]) + frame_t[b,c]
With q = exp(alpha - logZ_acc), E = exp(trans), F_t = exp(frame_t):
    q_{t+1} = F_t (.) (E^T q_t)
The E^T contraction runs on the PE (stationary bf16 weights, scaled by
2^-6 per step so magnitudes stay bounded; logZ gets the compile-time
constant T*6*ln2 back).  Exact per-batch normalization every 128 steps
(sum column from an extra ones-column in the weights) removes drift.

Layout: tags on partitions, batch on the free dim ([32, 128] per core).
Frames stream in as [128b, t*k], get exp'd on ACT (f32->bf16), transposed
to [t*k, b] by the PE, so the per-step DVE multiply
    q_{t+1} = s (.) F_t      (s = mm output in PSUM)
is a single [32, 128] tensor_tensor.

Sharding: pure batch data-parallel, 128 batch rows per NeuronCore x 8.
"""

import sys

import numpy as np

sys.path.insert(0, "/opt/trn_rl_repo")

import ml_dtypes

bf16 = ml_dtypes.bfloat16

B_TOT, T, K = 1024, 1024, 32
N_CORES = 8
B = B_TOT // N_CORES  # 128 per core
START_IX, END_IX = K - 2, K - 1
KSHIFT = 6  # per-step weight scale 2^-KSHIFT
NORM_EVERY = 128
CHUNK_T = 64  # time steps per frames DMA chunk

_cache = {}


def _build():
    import concourse.bass as bass
    import concourse.bacc as bacc
    import concourse.mybir as mybir
    import concourse.tile as tile

    f32 = mybir.dt.float32
    bf = mybir.dt.bfloat16

    nc = bacc.Bacc("TRN2")
    frames_d = nc.dram_tensor("frames", [B, T, K], f32, kind="ExternalInput").ap()
    wmat_d = nc.dram_tensor("wmat", [K, K + 1], bf, kind="ExternalInput").ap()
    eend_d = nc.dram_tensor("eend", [K, 1], bf, kind="ExternalInput").ap()
    ones_d = nc.dram_tensor("onesk", [1, K], f32, kind="ExternalInput").ap()
    q0_d = nc.dram_tensor("q0", [K, B], bf, kind="ExternalInput").ap()
    score_d = nc.dram_tensor("score", [1, B], f32, kind="ExternalOutput").ap()

    LOGZ_CONST = float(T * KSHIFT * np.log(2.0))
    NG = 2  # independent batch-group chains
    BG = B // NG  # 64 batches per chain

    with tile.TileContext(nc) as tc:
        with (
            tc.tile_pool(name="singles", bufs=1) as singles,
            tc.tile_pool(name="raw", bufs=3) as rawp,
            tc.tile_pool(name="expc", bufs=3) as expp,
            tc.tile_pool(name="fring", bufs=12) as fring,
            tc.tile_pool(name="qp", bufs=3) as qp,
            tc.tile_pool(name="psum_s", bufs=2, space="PSUM") as psum_s,
            tc.tile_pool(name="psum_s2", bufs=2, space="PSUM") as psum_s2,
            tc.tile_pool(name="psum_misc", bufs=1, space="PSUM") as psum_misc,
        ):
            # --- resident small tensors ---
            wmat = singles.tile([K, K + 1], bf)
            nc.sync.dma_start(wmat[:], wmat_d[:])
            eend = singles.tile([K, 1], bf)
            nc.sync.dma_start(eend[:], eend_d[:])
            onesk = singles.tile([1, K], f32)
            nc.sync.dma_start(onesk[:], ones_d[:])

            q_init = singles.tile([K, B], bf, name="q_init")
            nc.sync.dma_start(q_init[:], q0_d[:])

            n_norm = (T - 2) // NORM_EVERY  # norms at t=127,255,...,895
            c_hist = singles.tile([1, n_norm * B], f32)
            rc = singles.tile([1, B], f32)

            n_chunks = T // CHUNK_T
            f_tiles = [None] * (T // 4)

            def stage_chunk(c):
                """DMA + exp + dma-transpose frames chunk c (CHUNK_T steps)."""
                raw = rawp.tile([B, CHUNK_T * K], f32, tag="raw")
                nc.gpsimd.dma_start(raw[:], frames_d[:, c * CHUNK_T : (c + 1) * CHUNK_T, :])
                ex = expp.tile([B, CHUNK_T * K], bf, tag="ex")
                nc.scalar.activation(ex[:], raw[:], mybir.ActivationFunctionType.Exp)
                # transpose each 4-step group [128b, 128(t,k)] -> [128(t,k), 128b]
                for g in range(CHUNK_T // 4):
                    ft = fring.tile([128, B], bf, tag="ft")
                    nc.sync.dma_start_transpose(ft[:], ex[:, 128 * g : 128 * (g + 1)])
                    f_tiles[c * (CHUNK_T // 4) + g] = ft

            stage_chunk(0)
            qs = [q_init[:, gi * BG : (gi + 1) * BG] for gi in range(NG)]
            spools = [psum_s, psum_s2]

            for t in range(T):
                if t % CHUNK_T == 0 and t // CHUNK_T + 1 < n_chunks:
                    stage_chunk(t // CHUNK_T + 1)

                ft = f_tiles[t // 4]
                dt = t % 4
                # exact per-batch renorm at t=127,...,895: c (ones column of
                # this step's matmul) is folded into step t+2's F slice so the
                # correction stays off the q serial chain; ln(c) is deferred.
                is_norm = t % NORM_EVERY == NORM_EVERY - 1 and t + 2 < T
                norm_ix = t // NORM_EVERY

                for gi in range(NG):
                    s = spools[gi].tile([K + 1, BG], f32, tag=f"s{gi}")
                    nc.tensor.matmul(s[:], wmat[:], qs[gi])
                    fsl = ft[32 * dt : 32 * dt + 32, gi * BG : (gi + 1) * BG]
                    q_new = qp.tile([K, BG], bf, tag=f"q{gi}")
                    nc.vector.tensor_mul(q_new[:], s[0:K, :], fsl)
                    qs[gi] = q_new[:]
                    if is_norm:
                        lo = norm_ix * B + gi * BG
                        csl = c_hist[:, lo : lo + BG]
                        nc.vector.tensor_copy(csl, s[K : K + 1, :])
                        rsl = rc[:, gi * BG : (gi + 1) * BG]
                        nc.vector.reciprocal(rsl, csl)
                        rcb = psum_misc.tile([K, BG], f32, tag="rcb")
                        nc.tensor.matmul(rcb[:], onesk[:], rsl)
                        t2 = t + 2
                        ft2 = f_tiles[t2 // 4]
                        fsl2 = ft2[
                            32 * (t2 % 4) : 32 * (t2 % 4) + 32,
                            gi * BG : (gi + 1) * BG,
                        ]
                        nc.vector.tensor_mul(fsl2, fsl2, rcb[:])

            # logZ = const + sum_n ln(c_n), all deferred to the end
            lnc = singles.tile([1, n_norm * B], f32)
            nc.scalar.activation(lnc[:], c_hist[:], mybir.ActivationFunctionType.Ln)
            logz = singles.tile([1, B], f32)
            nc.vector.memset(logz[:], LOGZ_CONST)
            for n in range(n_norm):
                nc.vector.tensor_add(logz[:], logz[:], lnc[:, n * B : (n + 1) * B])

            fin = singles.tile([1, B], f32)
            for gi in range(NG):
                fin_ps = psum_misc.tile([1, BG], f32, tag="fin")
                nc.tensor.matmul(fin_ps[:], eend[:], qs[gi])
                nc.scalar.activation(
                    fin[:, gi * BG : (gi + 1) * BG],
                    fin_ps[:],
                    mybir.ActivationFunctionType.Ln,
                )
            out_sb = singles.tile([1, B], f32)
            nc.vector.tensor_add(out_sb[:], fin[:], logz[:])
            nc.sync.dma_start(score_d[:], out_sb[:])

    nc.compile()
    return nc


def _prep_aux(transitions):
    E = np.exp(transitions.astype(np.float64)) * (2.0 ** (-KSHIFT))
    wmat = np.ones((K, K + 1), dtype=np.float64)  # col K stays 1.0: c = sum_p q
    wmat[:, :K] = E
    eend = np.exp(transitions[:, END_IX].astype(np.float64)).reshape(K, 1)
    q0 = np.zeros((K, B), dtype=bf16)
    q0[START_IX, :] = 1.0
    return (
        wmat.astype(bf16),
        eend.astype(bf16),
        np.ones((1, K), dtype=np.float32),
        q0,
    )


def kernel(frames, transitions):
    from concourse.bass_utils import run_bass_kernel_spmd

    if "nc" not in _cache:
        _cache["nc"] = _build()
    nc = _cache["nc"]

    wmat, eend, onesk, q0 = _prep_aux(np.asarray(transitions))
    frames = np.ascontiguousarray(np.asarray(frames), dtype=np.float32)

    in_maps = []
    for i in range(N_CORES):
        in_maps.append(
            {
                "frames": frames[i * B : (i + 1) * B],
                "wmat": wmat,
                "eend": eend,
                "onesk": onesk,
                "q0": q0,
            }
        )
    res = run_bass_kernel_spmd(nc, in_maps, list(range(N_CORES)))
    out = np.concatenate([res.results[i]["score"][0] for i in range(N_CORES)])
    return out.astype(np.float32)


if __name__ == "__main__":
    rng = np.random.default_rng(0)
    fr = rng.standard_normal((B_TOT, T, K)).astype(np.float32)
    tr = rng.standard_normal((K, K)).astype(np.float32)
    tr[:, START_IX] = -10000.0
    tr[END_IX, :] = -10000.0
    out = kernel(fr, tr)
    print("kernel out:", out[:4], out.shape)
